# revision 25
# baseline (speedup 1.0000x reference)
"""Trainium2 Bass kernel for Euler-integrated Kuramoto dynamics.

    dtheta_i/dt = omega_i + sum_j K[i,j] * sin(theta_j - theta_i)

Strategy (8 NeuronCores, SPMD):
  sin(theta_j - theta_i) = sin(theta_j)cos(theta_i) - cos(theta_j)sin(theta_i)
so the per-step coupling reduction is two matvecs against K:
  coupling = cos(theta) * (K @ sin(theta)) - sin(theta) * (K @ cos(theta))

K is sharded row-wise: core c owns rows [512c, 512c+512). The shard is
staged as lhsT (K[rows,:].T, shape (4096, 512)) in fp16 and stays resident
in SBUF for all 50 steps (4 MB/core) — the matvec runs with K as the
stationary operand (fp16 => fast-weight-load) and a tiny (128, 2) moving
sin/cos operand. Each step every core updates its own 512 phases, computes
sin/cos of the updated shard (fp16, 2 KB), and exchanges it with the
other cores.

Exchange: per step the 2 KB own-shard sin/cos staging tile is dumped
p-major to DRAM (one block DMA), AllGathered across the 8 cores via the
collectives engine, and gathered back into the double-buffered full
sin/cos tile with one 3D block DMA.  (A remote_dma_broadcast SBUF->SBUF
exchange was prototyped and is ~15 us/step cheaper on paper, but the
SWDGE desc-gen instructions fault this runtime's gpsimd ucode, and
gpsimd register loads crash the worker, so the collective path stays.)

All SBUF layouts pack the 4096-vector as (128 partitions, 32 cols) with
element g = 128*col + p, so shard blocks are contiguous 8-col groups and
every access pattern is static.

Scalar-engine Sin is only valid on [-pi, pi]; phases drift outside, so
inputs are range-reduced with f = u - round(u) in turns-of-2pi and
sin = Sin(f * 2pi) via the activation scale.
"""

import os as _os

import numpy as np

N = 4096
M = 8  # cores
S = N // M  # 512 phases per core
NT = N // 128  # 32 contraction k-tiles
IT = S // 128  # 4 output i-tiles per core

N_STEPS = int(_os.environ.get("KUR_STEPS", "50"))
FLIP = bool(int(_os.environ.get("KUR_FLIP", "1")))
NO_MM = bool(int(_os.environ.get("KUR_NO_MM", "0")))
NO_CC = bool(int(_os.environ.get("KUR_NO_CC", "0")))
NO_DMA = bool(int(_os.environ.get("KUR_NO_DMA", "0")))
DT = 0.01
PI = 3.141592653589793

TRACE = False
LAST_RESULTS = None

_compiled_nc = None


def _build(n_steps=None, no_mm=NO_MM, no_cc=NO_CC, no_dma=NO_DMA):
    import concourse.bass as bass  # noqa: F401
    import concourse.tile as tile
    from concourse import bacc, mybir
    if n_steps is None:
        n_steps = N_STEPS

    f32 = mybir.dt.float32
    f16 = mybir.dt.float16
    AF = mybir.ActivationFunctionType
    OP = mybir.AluOpType

    nc = bacc.Bacc(
        "TRN2",
        target_bir_lowering=False,
        debug=False,
        enable_asserts=False,
        num_devices=M,
    )
    kt = nc.dram_tensor("kt", [N, S], f16, kind="ExternalInput").ap()
    ph = nc.dram_tensor("ph", [N], f32, kind="ExternalInput").ap()
    th0 = nc.dram_tensor("th0", [S], f32, kind="ExternalInput").ap()
    om = nc.dram_tensor("om", [S], f32, kind="ExternalInput").ap()  # dt*omega shard
    if FLIP:
        e2 = nc.dram_tensor("e2", [128, 2], f16, kind="ExternalInput").ap()
    th_out = nc.dram_tensor("th_out", [S], f32, kind="ExternalOutput").ap()

    with tile.TileContext(nc) as tc:
        with (
            tc.tile_pool(name="pers", bufs=1) as pers,
            tc.tile_pool(name="psum", bufs=2, space="PSUM") as psum_pool,
            tc.tile_pool(name="work", bufs=2) as work,
            tc.tile_pool(name="dram", bufs=2, space="DRAM") as dram,
        ):
            KT = pers.tile([128, NT * S], f16)  # k-tile t at cols [t*512,(t+1)*512)
            # full sin/cos, double-buffered by step parity:
            # col 2t = sin of k-tile t, col 2t+1 = cos of k-tile t
            SCs = [pers.tile([128, 2 * NT], f16, name=f"SC{i}") for i in range(2)]
            # own-shard staging for broadcast, interleaved sin/cos, 2 buffers
            SCos = [pers.tile([128, 2 * IT], f16, name=f"SCo{i}") for i in range(2)]
            T = pers.tile([128, IT], f32)  # own theta shard
            OM = pers.tile([128, IT], f32)  # dt*omega shard
            if FLIP:
                # chain-reduce/transpose selector: E2[32q+h, h] = 1 (host input
                # -- DVE memsets cannot start at non-32-aligned partitions)
                E2 = pers.tile([128, 2], f16)
                nc.sync.dma_start(E2[:], e2)
                # 32-col-padded stationary (sin_t, cos_t, 30 zeros per k-tile):
                # a 32-wide stationary makes each column-tiled chain write its
                # full 32-partition PSUM quadrant (zeros beyond row 1), so the
                # later full-tile PSUM->SBUF copy reads no uninitialized rows.
                # Live cols are refreshed from SC by one strided DVE copy.
                SCPs = [
                    pers.tile([128, 32 * NT], f16, name=f"SCP{i}") for i in range(2)
                ]
                nc.vector.memset(SCPs[0][:], 0.0)
                nc.vector.memset(SCPs[1][:], 0.0)

                def scp_refresh(q):
                    nc.vector.tensor_copy(
                        SCPs[q].rearrange("p (t e) -> p t e", e=32)[:, :, 0:2],
                        SCs[q].rearrange("p (t h) -> p t h", h=2),
                    )

            # --- preamble: K resident load + initial sin/cos of full phases ---
            for t in range(NT):
                nc.sync.dma_start(KT[:, t * S : (t + 1) * S], kt[t * 128 : (t + 1) * 128, :])
            nc.sync.dma_start(T[:], th0.rearrange("(a p) -> p a", p=128))
            nc.sync.dma_start(OM[:], om.rearrange("(a p) -> p a", p=128))
            T0f = work.tile([128, NT], f32, tag="t0f")
            nc.sync.dma_start(T0f[:], ph.rearrange("(q p) -> p q", p=128))

            INV2PI = 1.0 / (2.0 * PI)
            # (u + BIG) - BIG == round-to-nearest-integer(u) in fp32; the 1.5x
            # keeps u + BIG inside [2^23, 2^24) (ulp exactly 1) for negative u too
            BIG = 1.5 * 2.0**23

            def emit_sincos(dst_sin, dst_cos, src, shape_cols, tag):
                # Scalar-engine Sin is only valid on [-pi, pi]: reduce via
                # f = u - round(u) in turns-of-2pi, then Sin(f * 2pi).
                # Returns the first (sin) activation instruction as wait anchor.
                first_act = None
                for dst, quarter, nm in ((dst_sin, 0.0, "s"), (dst_cos, 0.25, "c")):
                    u = work.tile([128, shape_cols], f32, tag=f"u{nm}{tag}")
                    w = work.tile([128, shape_cols], f32, tag=f"w{nm}{tag}")
                    f = work.tile([128, shape_cols], f32, tag=f"f{nm}{tag}")
                    nc.vector.tensor_scalar(u[:], src, INV2PI, quarter, OP.mult, OP.add)
                    nc.vector.tensor_scalar(w[:], u[:], BIG, BIG, OP.add, OP.subtract)
                    nc.vector.tensor_tensor(f[:], u[:], w[:], OP.subtract)
                    act = nc.scalar.activation(dst, f[:], AF.Sin, scale=2.0 * PI)
                    if first_act is None:
                        first_act = act
                return first_act

            emit_sincos(SCs[0][:, 0::2], SCs[0][:, 1::2], T0f[:], NT, "f")
            emit_sincos(SCos[0][:, 0::2], SCos[0][:, 1::2], T[:], IT, "o")
            if FLIP:
                scp_refresh(0)

            for s in range(n_steps):
                p = s % 2
                SC = SCs[p]
                SCo = SCos[p]
                if FLIP:
                    # Flipped orientation: sin/cos is the (tiny, 2-col)
                    # stationary; the K shard streams as the 512-wide moving
                    # operand at one 128-elem column per cycle.  128x32 column
                    # tiling runs 4 independent k-tile chains concurrently
                    # (chain cq owns PSUM partitions 32cq..32cq+1), ~4x the
                    # matvec throughput of a single chain.
                    psf = psum_pool.tile([128, S], f32, tag="psf")
                    for j in range(NT // 4 if not no_mm else 1):
                        for cq in range(4):
                            t = cq * (NT // 4) + j
                            nc.tensor.matmul(
                                psf[32 * cq : 32 * cq + 32, :],
                                lhsT=SCPs[p][:, 32 * t : 32 * t + 32],
                                rhs=KT[:, t * S : (t + 1) * S],
                                start=(j == 0),
                                stop=(j == (NT // 4 - 1 if not no_mm else 0)),
                                tile_position=(0, 32 * cq),
                            )
                    # chain-reduce + transpose back to [128, 2*IT] via PE:
                    # ps2[n, h] = sum_r psf_sb[r, n] * E2[r, h] with
                    # E2[32q+h, h] = 1 sums the 4 partial A rows (h=0) and the
                    # 4 partial B rows (h=1) while transposing n into the
                    # partition dim.  fp16 partials keep ~1e-3 abs error on
                    # O(3) sums — far inside the step tolerance.
                    psf_sb = work.tile([128, S], f16, tag="psf_sb")
                    nc.vector.tensor_copy(psf_sb[:], psf[:])
                    ps = psum_pool.tile([128, 2 * IT], f32)
                    for it in range(IT):
                        nc.tensor.matmul(
                            ps[:, 2 * it : 2 * it + 2],
                            lhsT=psf_sb[:, it * 128 : (it + 1) * 128],
                            rhs=E2[:],
                            start=True,
                            stop=True,
                        )
                else:
                    ps = psum_pool.tile([128, 2 * IT], f32)
                    for it in range(IT if not no_mm else 1):
                        base = it * 128
                        for t in range(NT if not no_mm else 1):
                            nc.tensor.matmul(
                                ps[:, 2 * it : 2 * it + 2],
                                lhsT=KT[:, t * S + base : t * S + base + 128],
                                rhs=SC[:, 2 * t : 2 * t + 2],  # {sin_t, cos_t}
                                start=(t == 0),
                                stop=(t == (NT - 1 if not no_mm else 0)),
                            )
                # coupling = cos_own * (K@sin) - sin_own * (K@cos);  T += dt*coupling + dt*omega
                a = work.tile([128, IT], f32, tag="a")
                b = work.tile([128, IT], f32, tag="b")
                d = work.tile([128, IT], f32, tag="d")
                tom = work.tile([128, IT], f32, tag="tom")
                nc.vector.tensor_tensor(a[:], SCo[:, 1::2], ps[:, 0::2], OP.mult)
                nc.vector.tensor_tensor(b[:], SCo[:, 0::2], ps[:, 1::2], OP.mult)
                nc.vector.tensor_tensor(d[:], a[:], b[:], OP.subtract)
                nc.vector.tensor_tensor(tom[:], T[:], OM[:], OP.add)
                nc.vector.scalar_tensor_tensor(T[:], d[:], DT, tom[:], OP.mult, OP.add)

                if s < n_steps - 1:
                    q = (s + 1) % 2
                    emit_sincos(SCos[q][:, 0::2], SCos[q][:, 1::2], T[:], IT, "o")
                    # p-major staging layout: cin element p*8 + e <- SCo[p, e]
                    # is one plain 2D block DMA; the AllGather concatenates
                    # rank blocks, and the gather-back is one 3D block DMA
                    # SC[p, 8c+e] <- cout[c*1024 + p*8 + e].  (The old p-minor
                    # layout cost ~18 us/step in element-granular descriptors.)
                    cin = dram.tile([2 * S], f16, tag="cin")
                    cout = dram.tile([2 * S * M], f16, tag="cout")
                    if not no_dma:
                        nc.sync.dma_start(
                            cin.rearrange("(p e) -> p e", p=128),
                            SCos[q][:],
                        )
                    if not no_cc:
                        nc.gpsimd.collective_compute(
                            "AllGather",
                            OP.bypass,
                            replica_groups=[list(range(M))],
                            ins=[cin.opt()],
                            outs=[cout.opt()],
                        )
                    if not no_dma:
                        nc.sync.dma_start(
                            SCs[q].rearrange("p (c e) -> p c e", c=M),
                            cout.rearrange("(c p e) -> p c e", c=M, p=128),
                        )
                    if FLIP:
                        scp_refresh(q)

            nc.sync.dma_start(th_out.rearrange("(a p) -> p a", p=128), T[:])

    nc.compile()
    return nc


def _get_nc():
    global _compiled_nc
    if _compiled_nc is None:
        _compiled_nc = _build()
    return _compiled_nc


def _e2_const():
    e2 = np.zeros((128, 2), dtype=np.float16)
    for cq in range(4):
        for h in range(2):
            e2[32 * cq + h, h] = 1.0
    return e2


def make_in_maps(inputs):
    phases = np.ascontiguousarray(np.asarray(inputs["phases"], dtype=np.float32))
    K = np.asarray(inputs["K"], dtype=np.float32)
    omegas = np.asarray(inputs["omegas"], dtype=np.float32)
    in_maps = []
    for c in range(M):
        sl = slice(c * S, (c + 1) * S)
        in_maps.append(
            {
                # lhsT[j, i_local] = K[i, j] for this core's rows i
                "kt": np.ascontiguousarray(K[sl, :].T).astype(np.float16),
                "ph": phases,
                "th0": np.ascontiguousarray(phases[sl]),
                "om": np.ascontiguousarray(DT * omegas[sl]).astype(np.float32),
                **({"e2": _e2_const()} if FLIP else {}),
            }
        )
    return in_maps


def kernel(phases, K, omegas):
    global LAST_RESULTS
    from concourse import bass_utils

    nc = _get_nc()
    in_maps = make_in_maps({"phases": phases, "K": K, "omegas": omegas})
    res = bass_utils.run_bass_kernel_spmd(
        nc, in_maps, core_ids=list(range(M)), trace=TRACE
    )
    LAST_RESULTS = res
    out = np.concatenate([res.results[c]["th_out"] for c in range(M)])
    return out.astype(np.float32)
